# revision 5
# baseline (speedup 1.0000x reference)
"""LinearSelfAttention kernel for TRN2 (8 NeuronCores, batch-parallel).

Computes out = H + (PH @ mask(H^T Q H)) / n per sample, re-associated as
    HtQ = H^T Q            [s, e]
    PHt = (P H)^T          [s, d]
    Ct  = HtQ[:n]^T PHt[:n]  [e, d]   (mask = drop s == n row)
    out = H + (Ct/n)^T H
which is O(n d^2) instead of O(n^2 d).

Sharding: data-parallel over batch, 2 samples per core, P/Q replicated.
Matmuls in bf16 (fp32 PSUM accumulate); the fp32 H is added in the
epilogue on DVE so the dominant H term stays exact.
"""

import sys

sys.path.insert(0, "/opt/trn_rl_repo")

import numpy as np
import ml_dtypes

B, D1, N1 = 16, 257, 2049  # batch, d+1, n+1
N = N1 - 1  # 2048
NCORES = 8
BPC = B // NCORES  # samples per core

# partition chunking of the 257-sized dims: (offset, size)
CH = [(0, 128), (128, 128), (256, 1)]
NT = N // 128  # 16 full s-tiles (s == 2048 row is masked off)
# t chunks for the final matmul free dim
TCH = [(i * 512, min(512, N1 - i * 512)) for i in range((N1 + 511) // 512)]

_cached = {}


def _build():
    import concourse.bass as bass
    import concourse.tile as tile
    from concourse import bacc, mybir
    from contextlib import ExitStack

    f32 = mybir.dt.float32
    bf16 = mybir.dt.bfloat16

    nc = bacc.Bacc("TRN2", target_bir_lowering=False, debug=False, num_devices=NCORES)

    H_d = nc.declare_dram_parameter("H", [BPC, D1, N1], f32, isOutput=False)
    Hb_d = nc.declare_dram_parameter("Hb", [BPC, D1, N1], bf16, isOutput=False)
    QP_d = nc.declare_dram_parameter("QP", [D1, 514], bf16, isOutput=False)
    Y_d = nc.declare_dram_parameter("Y", [BPC, D1, N1], f32, isOutput=True)

    with tile.TileContext(nc) as tc:
        with ExitStack() as ctx:
            const = ctx.enter_context(tc.tile_pool(name="const", bufs=1))
            hfp = ctx.enter_context(tc.tile_pool(name="hfp", bufs=2))
            hbp = ctx.enter_context(tc.tile_pool(name="hbp", bufs=2))
            sq = ctx.enter_context(tc.tile_pool(name="sq", bufs=2))
            ctp = ctx.enter_context(tc.tile_pool(name="ctp", bufs=2))
            yp = ctx.enter_context(tc.tile_pool(name="yp", bufs=4))

            # ---- input DMAs, spread across engine queues so the first
            # s-tile's operands land as early as possible
            qp = []
            for c, (off, sz) in enumerate(CH):
                t = const.tile([128, 514], bf16, tag=f"qp{c}", name=f"qp{c}")
                nc.sync.dma_start(t[:sz, :], QP_d[off : off + sz, :])
                qp.append(t)

            hf = [[None] * 3 for _ in range(BPC)]
            hb = [[None] * 3 for _ in range(BPC)]
            load_eng = {
                (0, 0): nc.sync,
                (0, 1): nc.scalar,
                (0, 2): nc.gpsimd,
                (1, 0): nc.sync,
                (1, 1): nc.scalar,
                (1, 2): nc.gpsimd,
            }
            for b in range(BPC):
                for c, (off, sz) in enumerate(CH):
                    tb = hbp.tile([128, N1], bf16, tag=f"hb{c}", name=f"hb{b}_{c}")
                    load_eng[(b, c)].dma_start(tb[:sz, :], Hb_d[b, off : off + sz, :])
                    hb[b][c] = tb
            # fp32 H is only needed by the S4 epilogue; background queue
            for b in range(BPC):
                for c, (off, sz) in enumerate(CH):
                    tf = hfp.tile([128, N1], f32, tag=f"hf{c}", name=f"hf{b}_{c}")
                    nc.gpsimd.dma_start(tf[:sz, :], H_d[b, off : off + sz, :])
                    hf[b][c] = tf

            # ---- PE warmup: dummy matmuls bridge the input-DMA latency and
            # push the HAM clock gate to K=8/8. Results never read.
            wsb = const.tile([128, 128], bf16, tag="wsb", name="wsb")
            nc.vector.memset(wsb[:, :], 0.0)
            with tc.tile_pool(name="wp", bufs=1, space="PSUM") as wp:
                wps = wp.tile([128, 512], f32, tag="wps", name="warm_ps")
                NWARM = 80
                for i in range(NWARM):
                    nc.tensor.matmul(
                        wps[:, 0:128],
                        wsb[:, :],
                        wsb[:, :],
                        start=(i == 0),
                        stop=(i == NWARM - 1),
                    )

            htq = [None] * BPC
            pht = [None] * BPC
            # ---- S1 + S2: HtQ [s,e] and PHt [s,d], 16 s-tiles each ----
            with tc.tile_pool(name="pp12", bufs=3, space="PSUM") as pp12:
                for b in range(BPC):
                    htq[b] = sq.tile([128, NT * 257], bf16, tag="htq", name=f"htq{b}")
                    pht[b] = sq.tile([128, NT * 257], bf16, tag="pht", name=f"pht{b}")
                    for st in range(NT):
                        p_htq = pp12.tile(
                            [128, 257], f32, tag="p_htq", name=f"p_htq{b}_{st}"
                        )
                        p_pht = pp12.tile(
                            [128, 257], f32, tag="p_pht", name=f"p_pht{b}_{st}"
                        )
                        sl = slice(st * 128, (st + 1) * 128)
                        for c, (off, sz) in enumerate(CH):
                            st_flags = dict(start=(c == 0), stop=(c == 2))
                            nc.tensor.matmul(
                                p_htq[:, :],
                                hb[b][c][:sz, sl],
                                qp[c][:sz, 0:257],
                                **st_flags,
                            )
                            nc.tensor.matmul(
                                p_pht[:, :],
                                hb[b][c][:sz, sl],
                                qp[c][:sz, 257:514],
                                **st_flags,
                            )
                        osl = slice(st * 257, (st + 1) * 257)
                        # alternate eviction engines to balance DVE/ACT
                        if st % 2 == 0:
                            nc.vector.tensor_copy(htq[b][:, osl], p_htq[:, :])
                            nc.scalar.copy(pht[b][:, osl], p_pht[:, :])
                        else:
                            nc.scalar.copy(htq[b][:, osl], p_htq[:, :])
                            nc.vector.tensor_copy(pht[b][:, osl], p_pht[:, :])

            # ---- S3: Ct[e,d] = sum_{s<2048} HtQ[s,e] * PHt[s,d], scaled 1/n
            ct = [[None] * 3 for _ in range(BPC)]
            with tc.tile_pool(name="pp3", bufs=3, space="PSUM") as pp3:
                for b in range(BPC):
                    for ec, (eoff, esz) in enumerate(CH):
                        p_ct = pp3.tile([128, 257], f32, tag="p_ct", name=f"p_ct{b}_{ec}")
                        for st in range(NT):
                            base = st * 257
                            nc.tensor.matmul(
                                p_ct[:esz, :],
                                htq[b][:, base + eoff : base + eoff + esz],
                                pht[b][:, base : base + 257],
                                start=(st == 0),
                                stop=(st == NT - 1),
                            )
                        t = ctp.tile([128, 257], bf16, tag=f"ct{ec}", name=f"ct{b}_{ec}")
                        nc.scalar.mul(t[:esz, :], p_ct[:esz, :], 1.0 / N)
                        ct[b][ec] = t

            # ---- S4: Y = H + (Ct/n)^T H ----
            with tc.tile_pool(name="pp4", bufs=4, space="PSUM") as pp4:
                for b in range(BPC):
                    for dc, (doff, dsz) in enumerate(CH):
                        y = yp.tile([128, N1], f32, tag="y", name=f"y{b}_{dc}")
                        for toff, tsz in TCH:
                            p_a = pp4.tile(
                                [128, 512], f32, tag="p_a", name=f"p_a{b}_{dc}_{toff}"
                            )
                            for ec, (eoff, esz) in enumerate(CH):
                                nc.tensor.matmul(
                                    p_a[:dsz, :tsz],
                                    ct[b][ec][:esz, doff : doff + dsz],
                                    hb[b][ec][:esz, toff : toff + tsz],
                                    start=(ec == 0),
                                    stop=(ec == 2),
                                )
                            nc.vector.tensor_add(
                                y[:dsz, toff : toff + tsz],
                                p_a[:dsz, :tsz],
                                hf[b][dc][:dsz, toff : toff + tsz],
                            )
                            # store each chunk as soon as its epilogue add is done
                            nc.sync.dma_start(
                                Y_d[b, doff : doff + dsz, toff : toff + tsz],
                                y[:dsz, toff : toff + tsz],
                            )

    nc.compile()
    return nc


def _prep_in_maps(H, P, Q):
    H = np.ascontiguousarray(H, dtype=np.float32)
    Hb = H.astype(ml_dtypes.bfloat16)
    QP = np.ascontiguousarray(
        np.concatenate([Q, P.T], axis=1).astype(ml_dtypes.bfloat16)
    )
    return [
        {
            "H": H[c * BPC : (c + 1) * BPC],
            "Hb": Hb[c * BPC : (c + 1) * BPC],
            "QP": QP,
        }
        for c in range(NCORES)
    ]


def kernel(H, P, Q):
    from concourse.bass_utils import run_bass_kernel_spmd

    if "nc" not in _cached:
        _cached["nc"] = _build()
    nc = _cached["nc"]

    in_maps = _prep_in_maps(H, P, Q)
    res = run_bass_kernel_spmd(nc, in_maps, list(range(NCORES)))
    out = np.concatenate([res.results[c]["Y"] for c in range(NCORES)], axis=0)
    return out.astype(np.float32)


# revision 7
# speedup vs baseline: 1.0083x; 1.0083x over previous
"""LinearSelfAttention kernel for TRN2 (8 NeuronCores, batch-parallel).

Computes out = H + (PH @ mask(H^T Q H)) / n per sample, re-associated as
    HtQ = H^T Q            [s, e]
    PHt = (P H)^T          [s, d]
    Ct  = HtQ[:n]^T PHt[:n]  [e, d]   (mask = drop s == n row)
    out = H + (Ct/n)^T H
which is O(n d^2) instead of O(n^2 d).

Sharding: data-parallel over batch, 2 samples per core, P/Q replicated.
Matmuls in bf16 (fp32 PSUM accumulate); the fp32 H is added in the
epilogue on DVE so the dominant H term stays exact.
"""

import sys

sys.path.insert(0, "/opt/trn_rl_repo")

import numpy as np
import ml_dtypes

B, D1, N1 = 16, 257, 2049  # batch, d+1, n+1
N = N1 - 1  # 2048
NCORES = 8
BPC = B // NCORES  # samples per core

# partition chunking of the 257-sized dims: (offset, size)
CH = [(0, 128), (128, 128), (256, 1)]
NT = N // 128  # 16 full s-tiles (s == 2048 row is masked off)
# t chunks for the final matmul free dim
TCH = [(i * 512, min(512, N1 - i * 512)) for i in range((N1 + 511) // 512)]

_cached = {}


def _build():
    import concourse.bass as bass
    import concourse.tile as tile
    from concourse import bacc, mybir
    from contextlib import ExitStack

    f32 = mybir.dt.float32
    bf16 = mybir.dt.bfloat16

    nc = bacc.Bacc("TRN2", target_bir_lowering=False, debug=False, num_devices=NCORES)

    H_d = nc.declare_dram_parameter("H", [BPC, D1, N1], f32, isOutput=False)
    Hb_d = nc.declare_dram_parameter("Hb", [BPC, D1, N1], bf16, isOutput=False)
    QP_d = nc.declare_dram_parameter("QP", [D1, 514], bf16, isOutput=False)
    Y_d = nc.declare_dram_parameter("Y", [BPC, D1, N1], f32, isOutput=True)

    with tile.TileContext(nc) as tc:
        with ExitStack() as ctx:
            const = ctx.enter_context(tc.tile_pool(name="const", bufs=1))
            hfp = ctx.enter_context(tc.tile_pool(name="hfp", bufs=2))
            hbp = ctx.enter_context(tc.tile_pool(name="hbp", bufs=2))
            sq = ctx.enter_context(tc.tile_pool(name="sq", bufs=2))
            ctp = ctx.enter_context(tc.tile_pool(name="ctp", bufs=2))
            yp = ctx.enter_context(tc.tile_pool(name="yp", bufs=4))

            # ---- input DMAs, spread across engine queues so the first
            # s-tile's operands land as early as possible
            qp = []
            for c, (off, sz) in enumerate(CH):
                t = const.tile([128, 514], bf16, tag=f"qp{c}", name=f"qp{c}")
                nc.sync.dma_start(t[:sz, :], QP_d[off : off + sz, :])
                qp.append(t)

            hf = [[None] * 3 for _ in range(BPC)]
            hb = [[None] * 3 for _ in range(BPC)]
            load_eng = {0: nc.sync, 1: nc.scalar, 2: nc.gpsimd}
            for b in range(BPC):
                for c, (off, sz) in enumerate(CH):
                    tb = hbp.tile([128, N1], bf16, tag=f"hb{c}", name=f"hb{b}_{c}")
                    load_eng[c].dma_start(tb[:sz, :], Hb_d[b, off : off + sz, :])
                    hb[b][c] = tb
            # fp32 H is only needed by the S4 epilogue. Delay its issue until
            # the (urgent) bf16 loads have landed: a tiny gpsimd copy that
            # reads hb[0][0] keeps the hf dma_starts queued behind it.
            probe = const.tile([128, 16], bf16, tag="probe", name="probe")
            nc.gpsimd.tensor_copy(probe[0:1, :], hb[0][0][0:1, 0:16])
            for b in range(BPC):
                for c, (off, sz) in enumerate(CH):
                    tf = hfp.tile([128, N1], f32, tag=f"hf{c}", name=f"hf{b}_{c}")
                    nc.gpsimd.dma_start(tf[:sz, :], H_d[b, off : off + sz, :])
                    hf[b][c] = tf

            # ---- PE warmup: dummy matmuls bridge the input-DMA latency and
            # push the HAM clock gate to K=8/8. Results never read.
            wsb = const.tile([128, 128], bf16, tag="wsb", name="wsb")
            nc.vector.memset(wsb[:, :], 0.0)
            with tc.tile_pool(name="wp", bufs=1, space="PSUM") as wp:
                wps = wp.tile([128, 512], f32, tag="wps", name="warm_ps")
                NWARM = 110
                for i in range(NWARM):
                    nc.tensor.matmul(
                        wps[:, 0:128],
                        wsb[:, :],
                        wsb[:, :],
                        start=(i == 0),
                        stop=(i == NWARM - 1),
                    )

            htq = [None] * BPC
            pht = [None] * BPC
            # ---- S1 + S2: HtQ [s,e] and PHt [s,d], 16 s-tiles each ----
            with tc.tile_pool(name="pp12", bufs=3, space="PSUM") as pp12:
                for b in range(BPC):
                    htq[b] = sq.tile([128, NT * 257], bf16, tag="htq", name=f"htq{b}")
                    pht[b] = sq.tile([128, NT * 257], bf16, tag="pht", name=f"pht{b}")
                    for st in range(NT):
                        p_htq = pp12.tile(
                            [128, 257], f32, tag="p_htq", name=f"p_htq{b}_{st}"
                        )
                        p_pht = pp12.tile(
                            [128, 257], f32, tag="p_pht", name=f"p_pht{b}_{st}"
                        )
                        sl = slice(st * 128, (st + 1) * 128)
                        for c, (off, sz) in enumerate(CH):
                            st_flags = dict(start=(c == 0), stop=(c == 2))
                            nc.tensor.matmul(
                                p_htq[:, :],
                                hb[b][c][:sz, sl],
                                qp[c][:sz, 0:257],
                                **st_flags,
                            )
                            nc.tensor.matmul(
                                p_pht[:, :],
                                hb[b][c][:sz, sl],
                                qp[c][:sz, 257:514],
                                **st_flags,
                            )
                        osl = slice(st * 257, (st + 1) * 257)
                        # alternate eviction engines to balance DVE/ACT
                        if st % 2 == 0:
                            nc.vector.tensor_copy(htq[b][:, osl], p_htq[:, :])
                            nc.scalar.copy(pht[b][:, osl], p_pht[:, :])
                        else:
                            nc.scalar.copy(htq[b][:, osl], p_htq[:, :])
                            nc.vector.tensor_copy(pht[b][:, osl], p_pht[:, :])

            # ---- S3: Ct[e,d] = sum_{s<2048} HtQ[s,e] * PHt[s,d], scaled 1/n
            ct = [[None] * 3 for _ in range(BPC)]
            with tc.tile_pool(name="pp3", bufs=3, space="PSUM") as pp3:
                for b in range(BPC):
                    for ec, (eoff, esz) in enumerate(CH):
                        p_ct = pp3.tile([128, 257], f32, tag="p_ct", name=f"p_ct{b}_{ec}")
                        for st in range(NT):
                            base = st * 257
                            nc.tensor.matmul(
                                p_ct[:esz, :],
                                htq[b][:, base + eoff : base + eoff + esz],
                                pht[b][:, base : base + 257],
                                start=(st == 0),
                                stop=(st == NT - 1),
                            )
                        t = ctp.tile([128, 257], bf16, tag=f"ct{ec}", name=f"ct{b}_{ec}")
                        nc.scalar.mul(t[:esz, :], p_ct[:esz, :], 1.0 / N)
                        ct[b][ec] = t

            # ---- S4: Y = H + (Ct/n)^T H ----
            with tc.tile_pool(name="pp4", bufs=4, space="PSUM") as pp4:
                for b in range(BPC):
                    for dc, (doff, dsz) in enumerate(CH):
                        y = yp.tile([128, N1], f32, tag="y", name=f"y{b}_{dc}")
                        for toff, tsz in TCH:
                            p_a = pp4.tile(
                                [128, 512], f32, tag="p_a", name=f"p_a{b}_{dc}_{toff}"
                            )
                            for ec, (eoff, esz) in enumerate(CH):
                                nc.tensor.matmul(
                                    p_a[:dsz, :tsz],
                                    ct[b][ec][:esz, doff : doff + dsz],
                                    hb[b][ec][:esz, toff : toff + tsz],
                                    start=(ec == 0),
                                    stop=(ec == 2),
                                )
                            nc.vector.tensor_add(
                                y[:dsz, toff : toff + tsz],
                                p_a[:dsz, :tsz],
                                hf[b][dc][:dsz, toff : toff + tsz],
                            )
                            # store each chunk as soon as its epilogue add is done
                            nc.sync.dma_start(
                                Y_d[b, doff : doff + dsz, toff : toff + tsz],
                                y[:dsz, toff : toff + tsz],
                            )

    nc.compile()
    return nc


def _prep_in_maps(H, P, Q):
    H = np.ascontiguousarray(H, dtype=np.float32)
    Hb = H.astype(ml_dtypes.bfloat16)
    QP = np.ascontiguousarray(
        np.concatenate([Q, P.T], axis=1).astype(ml_dtypes.bfloat16)
    )
    return [
        {
            "H": H[c * BPC : (c + 1) * BPC],
            "Hb": Hb[c * BPC : (c + 1) * BPC],
            "QP": QP,
        }
        for c in range(NCORES)
    ]


def kernel(H, P, Q):
    from concourse.bass_utils import run_bass_kernel_spmd

    if "nc" not in _cached:
        _cached["nc"] = _build()
    nc = _cached["nc"]

    in_maps = _prep_in_maps(H, P, Q)
    res = run_bass_kernel_spmd(nc, in_maps, list(range(NCORES)))
    out = np.concatenate([res.results[c]["Y"] for c in range(NCORES)], axis=0)
    return out.astype(np.float32)


# revision 10
# speedup vs baseline: 1.0302x; 1.0217x over previous
"""LinearSelfAttention kernel for TRN2 (8 NeuronCores, batch-parallel).

Computes out = H + (PH @ mask(H^T Q H)) / n per sample, re-associated as
    HtQ = H^T Q            [s, e]
    PHt = (P H)^T          [s, d]
    Ct  = HtQ[:n]^T PHt[:n]  [e, d]   (mask = drop s == n row)
    out = H + (Ct/n)^T H
which is O(n d^2) instead of O(n^2 d).

Sharding: data-parallel over batch, 2 samples per core, P/Q replicated.
Matmuls in bf16 (fp32 PSUM accumulate); the fp32 H is added in the
epilogue on DVE so the dominant H term stays exact.
"""

import sys

sys.path.insert(0, "/opt/trn_rl_repo")

import numpy as np
import ml_dtypes

B, D1, N1 = 16, 257, 2049  # batch, d+1, n+1
N = N1 - 1  # 2048
NCORES = 8
BPC = B // NCORES  # samples per core

# partition chunking of the 257-sized dims: (offset, size)
CH = [(0, 128), (128, 128), (256, 1)]
NT = N // 128  # 16 full s-tiles (s == 2048 row is masked off)
# t chunks for the final matmul free dim
TCH = [(i * 512, min(512, N1 - i * 512)) for i in range((N1 + 511) // 512)]

_cached = {}


def _build():
    import concourse.bass as bass
    import concourse.tile as tile
    from concourse import bacc, mybir
    from contextlib import ExitStack

    f32 = mybir.dt.float32
    bf16 = mybir.dt.bfloat16

    nc = bacc.Bacc("TRN2", target_bir_lowering=False, debug=False, num_devices=NCORES)

    H_d = nc.declare_dram_parameter("H", [BPC, D1, N1], f32, isOutput=False)
    Hb_d = nc.declare_dram_parameter("Hb", [BPC, D1, N1], bf16, isOutput=False)
    QP_d = nc.declare_dram_parameter("QP", [D1, 514], bf16, isOutput=False)
    Y_d = nc.declare_dram_parameter("Y", [BPC, D1, N1], f32, isOutput=True)

    with tile.TileContext(nc) as tc:
        with ExitStack() as ctx:
            const = ctx.enter_context(tc.tile_pool(name="const", bufs=1))
            hfp = ctx.enter_context(tc.tile_pool(name="hfp", bufs=2))
            hbp = ctx.enter_context(tc.tile_pool(name="hbp", bufs=2))
            sq = ctx.enter_context(tc.tile_pool(name="sq", bufs=2))
            ctp = ctx.enter_context(tc.tile_pool(name="ctp", bufs=2))
            yp = ctx.enter_context(tc.tile_pool(name="yp", bufs=6))

            # ---- input DMAs, spread across engine queues so the first
            # s-tile's operands land as early as possible
            qp = []
            for c, (off, sz) in enumerate(CH):
                t = const.tile([128, 514], bf16, tag=f"qp{c}", name=f"qp{c}")
                nc.sync.dma_start(t[:sz, :], QP_d[off : off + sz, :])
                qp.append(t)

            hf = [[None] * 3 for _ in range(BPC)]
            hb = [[None] * 3 for _ in range(BPC)]
            # Priority class 0: sample-0 bf16 H (gates the first matmuls).
            # One tile per queue so it gets the full HBM bandwidth.
            load_eng = {0: nc.sync, 1: nc.scalar, 2: nc.gpsimd}
            for c, (off, sz) in enumerate(CH):
                tb = hbp.tile([128, N1], bf16, tag=f"hb{c}", name=f"hb0_{c}")
                load_eng[c].dma_start(tb[:sz, :], Hb_d[0, off : off + sz, :])
                hb[0][c] = tb
            # Priority class 1+2 (gpsimd, gated behind class 0 by probe
            # copies): sample-1 bf16 H, then the fp32 H for the epilogue.
            probe = const.tile([128, 16], bf16, tag="probe", name="probe")
            nc.gpsimd.tensor_copy(probe[0:1, 0:8], hb[0][0][0:1, 0:8])
            nc.gpsimd.tensor_copy(probe[0:1, 8:16], hb[0][1][0:1, 0:8])
            for c, (off, sz) in enumerate(CH):
                tb = hbp.tile([128, N1], bf16, tag=f"hb{c}", name=f"hb1_{c}")
                nc.gpsimd.dma_start(tb[:sz, :], Hb_d[1, off : off + sz, :])
                hb[1][c] = tb
            for b in range(BPC):
                for c, (off, sz) in enumerate(CH):
                    tf = hfp.tile([128, N1], f32, tag=f"hf{c}", name=f"hf{b}_{c}")
                    nc.gpsimd.dma_start(tf[:sz, :], H_d[b, off : off + sz, :])
                    hf[b][c] = tf

            # ---- PE warmup: dummy matmuls bridge the input-DMA latency and
            # push the HAM clock gate to K=8/8. Results never read.
            wsb = const.tile([128, 128], bf16, tag="wsb", name="wsb")
            nc.vector.memset(wsb[:, :], 0.0)
            with tc.tile_pool(name="wp", bufs=1, space="PSUM") as wp:
                wps = wp.tile([128, 512], f32, tag="wps", name="warm_ps")
                NWARM = 100
                for i in range(NWARM):
                    nc.tensor.matmul(
                        wps[:, 0:128],
                        wsb[:, :],
                        wsb[:, :],
                        start=(i == 0),
                        stop=(i == NWARM - 1),
                    )

            htq = [None] * BPC
            pht = [None] * BPC
            # ---- S1 + S2: HtQ [s,e] and PHt [s,d], 16 s-tiles each ----
            with tc.tile_pool(name="pp12", bufs=3, space="PSUM") as pp12:
                for b in range(BPC):
                    htq[b] = sq.tile([128, NT * 257], bf16, tag="htq", name=f"htq{b}")
                    pht[b] = sq.tile([128, NT * 257], bf16, tag="pht", name=f"pht{b}")
                    for st in range(NT):
                        p_htq = pp12.tile(
                            [128, 257], f32, tag="p_htq", name=f"p_htq{b}_{st}"
                        )
                        p_pht = pp12.tile(
                            [128, 257], f32, tag="p_pht", name=f"p_pht{b}_{st}"
                        )
                        sl = slice(st * 128, (st + 1) * 128)
                        for c, (off, sz) in enumerate(CH):
                            st_flags = dict(start=(c == 0), stop=(c == 2))
                            nc.tensor.matmul(
                                p_htq[:, :],
                                hb[b][c][:sz, sl],
                                qp[c][:sz, 0:257],
                                **st_flags,
                            )
                            nc.tensor.matmul(
                                p_pht[:, :],
                                hb[b][c][:sz, sl],
                                qp[c][:sz, 257:514],
                                **st_flags,
                            )
                        osl = slice(st * 257, (st + 1) * 257)
                        # alternate eviction engines to balance DVE/ACT
                        if st % 2 == 0:
                            nc.vector.tensor_copy(htq[b][:, osl], p_htq[:, :])
                            nc.scalar.copy(pht[b][:, osl], p_pht[:, :])
                        else:
                            nc.scalar.copy(htq[b][:, osl], p_htq[:, :])
                            nc.vector.tensor_copy(pht[b][:, osl], p_pht[:, :])

            # ---- S3: Ct[e,d] = sum_{s<2048} HtQ[s,e] * PHt[s,d], scaled 1/n
            ct = [[None] * 3 for _ in range(BPC)]
            with tc.tile_pool(name="pp3", bufs=3, space="PSUM") as pp3:
                for b in range(BPC):
                    for ec, (eoff, esz) in enumerate(CH):
                        p_ct = pp3.tile([128, 257], f32, tag="p_ct", name=f"p_ct{b}_{ec}")
                        for st in range(NT):
                            base = st * 257
                            nc.tensor.matmul(
                                p_ct[:esz, :],
                                htq[b][:, base + eoff : base + eoff + esz],
                                pht[b][:, base : base + 257],
                                start=(st == 0),
                                stop=(st == NT - 1),
                            )
                        t = ctp.tile([128, 257], bf16, tag=f"ct{ec}", name=f"ct{b}_{ec}")
                        nc.scalar.mul(t[:esz, :], p_ct[:esz, :], 1.0 / N)
                        ct[b][ec] = t

            # ---- S4: Y = H + (Ct/n)^T H ----
            with tc.tile_pool(name="pp4", bufs=4, space="PSUM") as pp4:
                for b in range(BPC):
                    for dc, (doff, dsz) in enumerate(CH):
                        y = yp.tile([128, N1], f32, tag="y", name=f"y{b}_{dc}")
                        for toff, tsz in TCH:
                            p_a = pp4.tile(
                                [128, 512], f32, tag="p_a", name=f"p_a{b}_{dc}_{toff}"
                            )
                            for ec, (eoff, esz) in enumerate(CH):
                                nc.tensor.matmul(
                                    p_a[:dsz, :tsz],
                                    ct[b][ec][:esz, doff : doff + dsz],
                                    hb[b][ec][:esz, toff : toff + tsz],
                                    start=(ec == 0),
                                    stop=(ec == 2),
                                )
                            nc.vector.tensor_add(
                                y[:dsz, toff : toff + tsz],
                                p_a[:dsz, :tsz],
                                hf[b][dc][:dsz, toff : toff + tsz],
                            )
                            # store each chunk as soon as its epilogue add is
                            # done; alternate queues so store issue keeps up
                            st_eng = nc.sync if (toff // 512) % 2 == 0 else nc.scalar
                            st_eng.dma_start(
                                Y_d[b, doff : doff + dsz, toff : toff + tsz],
                                y[:dsz, toff : toff + tsz],
                            )

    nc.compile()
    return nc


def _prep_in_maps(H, P, Q):
    H = np.ascontiguousarray(H, dtype=np.float32)
    Hb = H.astype(ml_dtypes.bfloat16)
    QP = np.ascontiguousarray(
        np.concatenate([Q, P.T], axis=1).astype(ml_dtypes.bfloat16)
    )
    return [
        {
            "H": H[c * BPC : (c + 1) * BPC],
            "Hb": Hb[c * BPC : (c + 1) * BPC],
            "QP": QP,
        }
        for c in range(NCORES)
    ]


def kernel(H, P, Q):
    from concourse.bass_utils import run_bass_kernel_spmd

    if "nc" not in _cached:
        _cached["nc"] = _build()
    nc = _cached["nc"]

    in_maps = _prep_in_maps(H, P, Q)
    res = run_bass_kernel_spmd(nc, in_maps, list(range(NCORES)))
    out = np.concatenate([res.results[c]["Y"] for c in range(NCORES)], axis=0)
    return out.astype(np.float32)


# revision 11
# speedup vs baseline: 1.2903x; 1.2525x over previous
"""LinearSelfAttention kernel for TRN2 (8 NeuronCores, batch-parallel).

Computes out = H + (PH @ mask(H^T Q H)) / n per sample, re-associated as
    HtQ = H^T Q            [s, e]
    PHt = (P H)^T          [s, d]
    Ct  = HtQ[:n]^T PHt[:n]  [e, d]   (mask = drop s == n row)
    out = H + (Ct/n)^T H
which is O(n d^2) instead of O(n^2 d).

Sharding: data-parallel over batch, 2 samples per core, P/Q replicated.
Matmuls in bf16 (fp32 PSUM accumulate); the fp32 H is added in the
epilogue on DVE so the dominant H term stays exact.
"""

import sys

sys.path.insert(0, "/opt/trn_rl_repo")

import numpy as np
import ml_dtypes

B, D1, N1 = 16, 257, 2049  # batch, d+1, n+1
N = N1 - 1  # 2048
NCORES = 8
BPC = B // NCORES  # samples per core

# partition chunking of the 257-sized dims: (offset, size)
CH = [(0, 128), (128, 128), (256, 1)]
NT = N // 128  # 16 full s-tiles (s == 2048 row is masked off)
# t chunks for the final matmul free dim
TCH = [(i * 512, min(512, N1 - i * 512)) for i in range((N1 + 511) // 512)]

_cached = {}


def _build():
    import concourse.bass as bass
    import concourse.tile as tile
    from concourse import bacc, mybir
    from contextlib import ExitStack

    f32 = mybir.dt.float32
    bf16 = mybir.dt.bfloat16

    nc = bacc.Bacc("TRN2", target_bir_lowering=False, debug=False, num_devices=NCORES)

    H_d = nc.declare_dram_parameter("H", [BPC, D1, N1], f32, isOutput=False)
    Hb_d = nc.declare_dram_parameter("Hb", [BPC, D1, N1], bf16, isOutput=False)
    QP_d = nc.declare_dram_parameter("QP", [D1, 514], bf16, isOutput=False)
    Y_d = nc.declare_dram_parameter("Y", [BPC, D1, N1], f32, isOutput=True)

    with tile.TileContext(nc) as tc:
        with ExitStack() as ctx:
            const = ctx.enter_context(tc.tile_pool(name="const", bufs=1))
            hfp = ctx.enter_context(tc.tile_pool(name="hfp", bufs=2))
            hbp = ctx.enter_context(tc.tile_pool(name="hbp", bufs=2))
            sq = ctx.enter_context(tc.tile_pool(name="sq", bufs=2))
            ctp = ctx.enter_context(tc.tile_pool(name="ctp", bufs=2))
            yp = ctx.enter_context(tc.tile_pool(name="yp", bufs=6))

            # ---- input DMAs, spread across engine queues so the first
            # s-tile's operands land as early as possible
            qp = []
            for c, (off, sz) in enumerate(CH):
                t = const.tile([128, 514], bf16, tag=f"qp{c}", name=f"qp{c}")
                nc.sync.dma_start(t[:sz, :], QP_d[off : off + sz, :])
                qp.append(t)

            hf = [[None] * 3 for _ in range(BPC)]
            hb = [[None] * 3 for _ in range(BPC)]
            # Priority class 0: sample-0 bf16 H (gates the first matmuls).
            # One tile per queue so it gets the full HBM bandwidth.
            load_eng = {0: nc.sync, 1: nc.scalar, 2: nc.gpsimd}
            for c, (off, sz) in enumerate(CH):
                tb = hbp.tile([128, N1], bf16, tag=f"hb{c}", name=f"hb0_{c}")
                load_eng[c].dma_start(tb[:sz, :], Hb_d[0, off : off + sz, :])
                hb[0][c] = tb
            # Priority class 1+2 (gpsimd, gated behind class 0 by probe
            # copies): sample-1 bf16 H, then the fp32 H for the epilogue.
            probe = const.tile([128, 16], bf16, tag="probe", name="probe")
            nc.gpsimd.tensor_copy(probe[0:1, 0:8], hb[0][0][0:1, 0:8])
            nc.gpsimd.tensor_copy(probe[0:1, 8:16], hb[0][1][0:1, 0:8])
            for c, (off, sz) in enumerate(CH):
                tb = hbp.tile([128, N1], bf16, tag=f"hb{c}", name=f"hb1_{c}")
                nc.gpsimd.dma_start(tb[:sz, :], Hb_d[1, off : off + sz, :])
                hb[1][c] = tb
            for b in range(BPC):
                for c, (off, sz) in enumerate(CH):
                    tf = hfp.tile([128, N1], f32, tag=f"hf{c}", name=f"hf{b}_{c}")
                    nc.gpsimd.dma_start(tf[:sz, :], H_d[b, off : off + sz, :])
                    hf[b][c] = tf

            # ---- PE warmup: dummy matmuls bridge the input-DMA latency and
            # push the HAM clock gate to K=8/8. Results never read.
            wsb = const.tile([128, 128], bf16, tag="wsb", name="wsb")
            nc.vector.memset(wsb[:, :], 0.0)
            with tc.tile_pool(name="wp", bufs=1, space="PSUM") as wp:
                wps = wp.tile([128, 512], f32, tag="wps", name="warm_ps")
                NWARM = 145
                for i in range(NWARM):
                    nc.tensor.matmul(
                        wps[:, 0:128],
                        wsb[:, :],
                        wsb[:, :],
                        start=(i == 0),
                        stop=(i == NWARM - 1),
                    )

            htq = [None] * BPC
            pht = [None] * BPC
            # ---- S1 + S2: HtQ [s,e] and PHt [s,d], 16 s-tiles each ----
            with tc.tile_pool(name="pp12", bufs=3, space="PSUM") as pp12:
                for b in range(BPC):
                    htq[b] = sq.tile([128, NT * 257], bf16, tag="htq", name=f"htq{b}")
                    pht[b] = sq.tile([128, NT * 257], bf16, tag="pht", name=f"pht{b}")
                    for st in range(NT):
                        p_htq = pp12.tile(
                            [128, 257], f32, tag="p_htq", name=f"p_htq{b}_{st}"
                        )
                        p_pht = pp12.tile(
                            [128, 257], f32, tag="p_pht", name=f"p_pht{b}_{st}"
                        )
                        sl = slice(st * 128, (st + 1) * 128)
                        for c, (off, sz) in enumerate(CH):
                            st_flags = dict(start=(c == 0), stop=(c == 2))
                            nc.tensor.matmul(
                                p_htq[:, :],
                                hb[b][c][:sz, sl],
                                qp[c][:sz, 0:257],
                                **st_flags,
                            )
                            nc.tensor.matmul(
                                p_pht[:, :],
                                hb[b][c][:sz, sl],
                                qp[c][:sz, 257:514],
                                **st_flags,
                            )
                        osl = slice(st * 257, (st + 1) * 257)
                        # alternate eviction engines to balance DVE/ACT
                        if st % 2 == 0:
                            nc.vector.tensor_copy(htq[b][:, osl], p_htq[:, :])
                            nc.scalar.copy(pht[b][:, osl], p_pht[:, :])
                        else:
                            nc.scalar.copy(htq[b][:, osl], p_htq[:, :])
                            nc.vector.tensor_copy(pht[b][:, osl], p_pht[:, :])

            # ---- S3: Ct[e,d] = sum_{s<2048} HtQ[s,e] * PHt[s,d], scaled 1/n
            ct = [[None] * 3 for _ in range(BPC)]
            with tc.tile_pool(name="pp3", bufs=3, space="PSUM") as pp3:
                for b in range(BPC):
                    for ec, (eoff, esz) in enumerate(CH):
                        p_ct = pp3.tile([128, 257], f32, tag="p_ct", name=f"p_ct{b}_{ec}")
                        for st in range(NT):
                            base = st * 257
                            nc.tensor.matmul(
                                p_ct[:esz, :],
                                htq[b][:, base + eoff : base + eoff + esz],
                                pht[b][:, base : base + 257],
                                start=(st == 0),
                                stop=(st == NT - 1),
                            )
                        t = ctp.tile([128, 257], bf16, tag=f"ct{ec}", name=f"ct{b}_{ec}")
                        nc.scalar.mul(t[:esz, :], p_ct[:esz, :], 1.0 / N)
                        ct[b][ec] = t

            # ---- S4: Y = H + (Ct/n)^T H ----
            with tc.tile_pool(name="pp4", bufs=4, space="PSUM") as pp4:
                for b in range(BPC):
                    for dc, (doff, dsz) in enumerate(CH):
                        y = yp.tile([128, N1], f32, tag="y", name=f"y{b}_{dc}")
                        for toff, tsz in TCH:
                            p_a = pp4.tile(
                                [128, 512], f32, tag="p_a", name=f"p_a{b}_{dc}_{toff}"
                            )
                            for ec, (eoff, esz) in enumerate(CH):
                                nc.tensor.matmul(
                                    p_a[:dsz, :tsz],
                                    ct[b][ec][:esz, doff : doff + dsz],
                                    hb[b][ec][:esz, toff : toff + tsz],
                                    start=(ec == 0),
                                    stop=(ec == 2),
                                )
                            nc.vector.tensor_add(
                                y[:dsz, toff : toff + tsz],
                                p_a[:dsz, :tsz],
                                hf[b][dc][:dsz, toff : toff + tsz],
                            )
                            # store each chunk as soon as its epilogue add is
                            # done; alternate queues so store issue keeps up
                            st_eng = nc.sync if (toff // 512) % 2 == 0 else nc.scalar
                            st_eng.dma_start(
                                Y_d[b, doff : doff + dsz, toff : toff + tsz],
                                y[:dsz, toff : toff + tsz],
                            )

    nc.compile()
    return nc


def _prep_in_maps(H, P, Q):
    H = np.ascontiguousarray(H, dtype=np.float32)
    Hb = H.astype(ml_dtypes.bfloat16)
    QP = np.ascontiguousarray(
        np.concatenate([Q, P.T], axis=1).astype(ml_dtypes.bfloat16)
    )
    return [
        {
            "H": H[c * BPC : (c + 1) * BPC],
            "Hb": Hb[c * BPC : (c + 1) * BPC],
            "QP": QP,
        }
        for c in range(NCORES)
    ]


def kernel(H, P, Q):
    from concourse.bass_utils import run_bass_kernel_spmd

    if "nc" not in _cached:
        _cached["nc"] = _build()
    nc = _cached["nc"]

    in_maps = _prep_in_maps(H, P, Q)
    res = run_bass_kernel_spmd(nc, in_maps, list(range(NCORES)))
    out = np.concatenate([res.results[c]["Y"] for c in range(NCORES)], axis=0)
    return out.astype(np.float32)
